# revision 11
# baseline (speedup 1.0000x reference)
"""4-layer GAT on 8 Trainium2 NeuronCores (Bass/Tile, SPMD) — v2.

Sharding: nodes partitioned across 8 cores; edges sharded by destination so
segment-softmax and the weighted scatter-add stay local. Per layer each core
projects its own nodes and the per-node rows [h | alpha_src] are AllGathered
into a per-core DRAM table; the edge phase gathers source rows by index
(SWDGE dma_gather) in a degree-sorted, per-destination layout and segment-sums
via identity-lhsT matmuls accumulating in PSUM.

v2 changes vs v1:
- Layer-1 table is computed locally on every core from the replicated input x
  (projection at fin=128 is cheap), eliminating the largest AllGather.
- Gathers round-robin over 4 SWDGE queues (measured 1.7x gather throughput:
  single-queue dma_gather is descriptor-rate-bound at ~8 ns/slot).
- The next layer's projection is interleaved into the edge-phase block loop so
  the single AllGather per layer fires as early as possible.
- One dma_gather per destination block (up to JCAP slots) instead of 8-slot
  chunks: fewer SWDGE descriptor-generation stalls on the Pool engine.
- Feature columns are stored (c,h)-interleaved (head index fastest) so the
  alpha multiply p*h broadcasts alpha over c with h packed innermost; with
  all-fp16 operands the DVE runs this at 2 elem/cycle instead of 1.
- Layer-1 projection batches 4 vblocks per DMA (SP sequencer was congested by
  320 x 565 ns of dma_start issue time).
"""
import math
import numpy as np

import concourse.bass as bass
import concourse.bacc as bacc
import concourse.mybir as mybir
import concourse.tile as tile
from concourse import bass_utils

FP16 = mybir.dt.float16
FP8 = mybir.dt.float8e4
FP32 = mybir.dt.float32
I16 = mybir.dt.int16

N_CORES = 8
JCAP = 14        # max slots per gather chunk
BN_EPS = 1e-5
NEG_SLOPE = 0.2
AG8 = False      # AllGather in fp8 + local upconvert to the fp16 gather table
TB8 = False      # fp8 tables end-to-end: AG and gather both fp8, no convert
CVT = 4          # row-tiles per conversion DMA batch
NQ = 4           # SWDGE queues: gathers round-robin across them
GPB = 4          # gather-pool buffers
MPB = 2          # M-pool buffers


# ----------------------------------------------------------------- host prep

def _prep_graph(n, edge_index, npc, npad):
    """Edge structure -> per-core degree-sorted blocks + unified slot counts.

    Table rows are numbered chunk-major: vid = ch*(8*npad/2) + c*(npad/2) + i
    for own local position i (i within the chunk's half of the blocks).
    """
    e = edge_index
    src = np.concatenate([e[0].astype(np.int64), np.arange(n, dtype=np.int64)])
    dst = np.concatenate([e[1].astype(np.int64), np.arange(n, dtype=np.int64)])
    deg = np.bincount(dst, minlength=n)

    order = np.argsort(dst, kind="stable")
    src_sorted = src[order]
    starts = np.zeros(n + 1, np.int64)
    np.cumsum(deg, out=starts[1:])

    nblk = npad // 128
    order_deg = np.argsort(-deg, kind="stable")
    perms = [order_deg[c::N_CORES] for c in range(N_CORES)]

    g2v = np.full(n, -1, np.int64)
    for c in range(N_CORES):
        g2v[perms[c]] = c * npad + np.arange(npc)

    # unified per-block slot counts
    D = np.zeros(nblk, np.int64)
    for c in range(N_CORES):
        dsort = deg[perms[c]]
        for b in range(nblk):
            blk = dsort[b * 128:(b + 1) * 128]
            if len(blk):
                D[b] = max(D[b], int(blk.max()))
    D = np.maximum(D, 1)

    # gather chunk layout (same for all cores): per block, chunks of <=JCAP
    chunks = []         # (block, j0, jc, idx_col_off, mask_col_off)
    icol = 0
    mcol = 0
    for b in range(nblk):
        j0 = 0
        while j0 < D[b]:
            jc = min(JCAP, int(D[b]) - j0)
            chunks.append((b, j0, jc, icol, mcol + j0))
            icol += (128 * jc) // 16
            j0 += jc
        mcol += int(D[b])
    icols_total = icol
    sumd = mcol

    idx_imgs = np.zeros((N_CORES, 128, icols_total), np.int16)
    masks = np.zeros((N_CORES, 128, sumd), np.float16)
    for c in range(N_CORES):
        p = perms[c]
        mdcol = 0
        for b in range(nblk):
            for d in range(128):
                pos = b * 128 + d
                nd = int(deg[p[pos]]) if pos < npc else 0
                masks[c, d, mdcol + nd:mdcol + D[b]] = -30000.0
            mdcol += int(D[b])
        for (b, j0, jc, ic, _mc) in chunks:
            ni = 128 * jc
            flat = np.zeros(ni, np.int16)
            for d in range(128):
                pos = b * 128 + d
                if pos >= npc:
                    continue
                g = p[pos]
                nd = int(deg[g])
                s = starts[g]
                hi = min(j0 + jc, nd)
                for j in range(j0, hi):
                    flat[(j - j0) * 128 + d] = g2v[src_sorted[s + j]]
            img = flat.reshape(-1, 16).T  # [16, ni/16]
            for r in range(0, 128, 16):
                idx_imgs[c, r:r + 16, ic:ic + ni // 16] = img
    return dict(perms=perms, g2v=g2v, D=D, chunks=chunks, nblk=nblk,
                idx_imgs=idx_imgs, masks=masks, sumd=sumd, icols=icols_total)


def _fperm(f, h):
    """Permutation old->new for (c,h) feature order: new index c*H+h_i maps to
    old index h_i*C+c.  Returns array old_of_new[new] = old."""
    c = f // h
    old = np.empty(f, np.int64)
    for ci in range(c):
        for hi in range(h):
            old[ci * h + hi] = hi * c + ci
    return old


def _prep_layers(inputs):
    """Fold weights host-side ((c,h) feature order). List of per-layer dicts."""
    layers = []
    specs = [("1", 8), ("2", 7), ("3", 7), ("4", 5)]
    prev_perm = None  # feature permutation applied to previous layer's output
    for li, (s, H) in enumerate(specs):
        W = inputs["W" + s].astype(np.float64)
        a_s = inputs["as" + s].astype(np.float64)
        a_d = inputs["ad" + s].astype(np.float64)
        fin, fout = W.shape
        C = a_s.shape[1]
        if prev_perm is not None:
            W = W[prev_perm, :]
        perm = _fperm(fout, H)
        Wp = W[:, perm]
        Was = Wp @ _blockdiag_chp(a_s, fout, H)
        Wad = Wp @ _blockdiag_chp(a_d, fout, H)
        Wext = np.concatenate([Wp, Was, Wad], axis=1)  # [fin, F+2H]
        d = dict(fin=fin, F=fout, H=H, C=C, Wext=Wext.astype(np.float16))
        if li < 3:
            g = inputs["g" + s].astype(np.float64)[perm]
            be = inputs["be" + s].astype(np.float64)[perm]
            m = inputs["m" + s].astype(np.float64)[perm]
            v = inputs["v" + s].astype(np.float64)[perm]
            b = inputs["b" + s].astype(np.float64)[perm]
            sc = g / np.sqrt(v + BN_EPS)
            bias = (b - m) * sc + be
            d["bnscale"] = sc.astype(np.float32)
            d["bnbias"] = bias.astype(np.float32)
        else:
            d["b4"] = inputs["b" + s].astype(np.float64)
        d["R"] = ((fout + H + 127) // 128) * 128
        d["R8"] = ((fout + 2 * H + 255) // 256) * 256
        d["NW"] = fout + 2 * H
        layers.append(d)
        prev_perm = perm
    return layers


def _blockdiag_chp(a, f, h):
    """a: [H, C] -> [F x H] where row index is already (c,h)-ordered:
    A[c*H+hi, hi] = a[hi, c]."""
    hh, cc = a.shape
    assert hh == h and f == hh * cc
    out = np.zeros((f, hh), np.float64)
    for hi in range(hh):
        for c in range(cc):
            out[c * hh + hi, hi] = a[hi, c]
    return out


def _ktiles(fin):
    ks = []
    o = 0
    while o < fin:
        k = min(128, fin - o)
        ks.append((o, k))
        o += k
    return ks


def _ftiles(f):
    fs = []
    o = 0
    while o < f:
        w = min(128, f - o)
        fs.append((o, w))
        o += w
    return fs


# ------------------------------------------------------------- device build

def _build(n, npc, npad, layers, graph, pw, repeat=1, cost_mode=False, abl=()):
    nblk = graph["nblk"]
    V = N_CORES * npad
    chunks = graph["chunks"]
    D = graph["D"]
    sumd = graph["sumd"]
    icols = graph["icols"]
    nvblk = V // 128

    nc = bacc.Bacc("TRN2", target_bir_lowering=False, debug=False,
                   num_devices=1 if cost_mode else N_CORES,
                   num_swdge_queues=NQ)
    qctr = [0]

    # ---- IO
    xT_in = nc.dram_tensor("xT", [128, V], FP16, kind="ExternalInput").ap()
    xTo_in = nc.dram_tensor("xTo", [128, npad], FP16, kind="ExternalInput").ap()
    idx_in = nc.dram_tensor("idx", [128, icols], I16, kind="ExternalInput").ap()
    mask_in = nc.dram_tensor("mask", [128, sumd], FP16, kind="ExternalInput").ap()
    ident_in = nc.dram_tensor("ident", [128, 128], FP16, kind="ExternalInput").ap()
    ones_in = nc.dram_tensor("ones1", [1, 128], FP16, kind="ExternalInput").ap()
    b4_in = nc.dram_tensor("b4row", [1, layers[3]["NW"]], FP16, kind="ExternalInput").ap()
    w_in = []
    bn_in = []
    for li, L in enumerate(layers):
        wl = []
        for kt, (o, k) in enumerate(_ktiles(L["fin"])):
            wl.append(nc.dram_tensor(f"w{li}_{kt}", [k, L["NW"]], FP16,
                                     kind="ExternalInput").ap())
        w_in.append(wl)
        if li < 3:
            nft = len(_ftiles(L["F"]))
            bn_in.append((
                nc.dram_tensor(f"bnsc{li}", [128, nft], FP32, kind="ExternalInput").ap(),
                nc.dram_tensor(f"bnbi{li}", [128, nft], FP32, kind="ExternalInput").ap(),
            ))
        else:
            bn_in.append(None)
    out_dram = nc.dram_tensor("out", [npad, layers[3]["C"]], FP32, kind="ExternalOutput").ap()

    with tile.TileContext(nc) as tc:
        with tc.tile_pool(name="const", bufs=1) as cpool, \
             tc.tile_pool(name="work", bufs=3) as pool, \
             tc.tile_pool(name="gpool", bufs=GPB) as gpool, \
             tc.tile_pool(name="mpool", bufs=MPB) as mpool, \
             tc.tile_pool(name="cvtp", bufs=2) as cvtp, \
             tc.tile_pool(name="zt", bufs=1) as zpool, \
             tc.tile_pool(name="psA", bufs=2, space="PSUM") as ppA, \
             tc.tile_pool(name="psB", bufs=2, space="PSUM") as ppB, \
             tc.tile_pool(name="psagg", bufs=2, space="PSUM") as ppG, \
             tc.tile_pool(name="dram", bufs=1, space="DRAM") as dpool:

            # ---- persistent SBUF
            ident = cpool.tile([128, 128], FP16)
            nc.sync.dma_start(ident[:], ident_in[:])
            ones1 = cpool.tile([1, 128], FP16)
            nc.sync.dma_start(ones1[:], ones_in[:])
            b4row = cpool.tile([1, layers[3]["NW"]], FP16)
            nc.sync.dma_start(b4row[:], b4_in[:])
            idx_sb = cpool.tile([128, icols], I16)
            nc.sync.dma_start(idx_sb[:], idx_in[:])
            mask_sb = cpool.tile([128, sumd], FP16)
            nc.sync.dma_start(mask_sb[:], mask_in[:])
            w_sb = []
            bn_sb = []
            for li, L in enumerate(layers):
                wl = []
                for kt, (o, k) in enumerate(_ktiles(L["fin"])):
                    t = cpool.tile([k, L["NW"]], FP16, tag=f"w{li}_{kt}")
                    nc.sync.dma_start(t[:], w_in[li][kt][:])
                    wl.append(t)
                w_sb.append(wl)
                if li < 3:
                    nft = len(_ftiles(L["F"]))
                    s = cpool.tile([128, nft], FP32, tag=f"bs{li}")
                    bbt = cpool.tile([128, nft], FP32, tag=f"bb{li}")
                    nc.sync.dma_start(s[:], bn_in[li][0][:])
                    nc.sync.dma_start(bbt[:], bn_in[li][1][:])
                    bn_sb.append((s, bbt))
                else:
                    bn_sb.append(None)
            ad_own = cpool.tile([128, nblk * 8], FP32)  # alpha_dst own nodes

            for _rep in range(repeat):
              tables = []
              bounces = []
              table8s = []
              for li, L in enumerate(layers):
                  shared = (li > 0) and not AG8
                  if TB8:
                      tb = dpool.tile([V, L["R8"]], FP8, tag=f"table{li}",
                                      name=f"table{li}",
                                      addr_space="Shared" if li else "Local")
                  else:
                      tb = dpool.tile([V, L["R"]], FP16, tag=f"table{li}",
                                      name=f"table{li}",
                                      addr_space="Shared" if shared else "Local")
                  tables.append(tb)
                  if li == 0:
                      bounces.append(None)
                      table8s.append(None)
                  elif TB8:
                      bn_t = dpool.tile([npad, L["R8"]], FP8, tag=f"bounce{li}",
                                        name=f"bounce{li}")
                      bounces.append(bn_t)
                      table8s.append(None)
                  elif AG8:
                      bn_t = dpool.tile([npad, L["R8"]], FP8, tag=f"bounce{li}",
                                        name=f"bounce{li}")
                      bounces.append(bn_t)
                      t8 = dpool.tile([V, L["R8"]], FP8, tag=f"table8{li}",
                                      name=f"table8{li}", addr_space="Shared")
                      table8s.append(t8)
                  else:
                      bn_t = dpool.tile([npad, L["R"]], FP16, tag=f"bounce{li}",
                                        name=f"bounce{li}")
                      bounces.append(bn_t)
                      table8s.append(None)

              # ---------- layer-1 table: project ALL nodes locally (x replicated)
              L0 = layers[0]
              F0, H0, R0, NW0 = L0["F"], L0["H"], L0["R"], L0["NW"]
              XB = 4
              R80 = L0["R8"]
              RT0 = R80 if TB8 else R0
              for vb0 in range(0, nvblk, XB):
                  xt = pool.tile([128, XB * 128], FP16, tag="xt")
                  nc.sync.dma_start(xt[:], xT_in[:, vb0 * 128:(vb0 + XB) * 128])
                  ownb = pool.tile([128, XB, RT0], FP8 if TB8 else FP16,
                                   tag="own0")
                  for i in range(XB):
                      vb = vb0 + i
                      psA = ppA.tile([128, 512], FP32, tag="gA")
                      psB = ppB.tile([128, 128], FP32, tag="gB")
                      nc.tensor.matmul(psA[:, :512],
                                       xt[:, i * 128:(i + 1) * 128],
                                       w_sb[0][0][:, :512],
                                       start=True, stop=True)
                      nc.tensor.matmul(psB[:, :NW0 - 512],
                                       xt[:, i * 128:(i + 1) * 128],
                                       w_sb[0][0][:, 512:NW0],
                                       start=True, stop=True)
                      own = ownb[:, i, :]
                      if TB8:
                          nc.vector.memset(own[:, F0 + 2 * H0:R80], 0.0)
                          if vb % 2 == 0:
                              nc.scalar.copy(own[:, :512], psA[:, :512])
                              nc.scalar.copy(
                                  own[:, F0:F0 + 2 * H0].bitcast(FP16),
                                  psB[:, :H0])
                          else:
                              nc.vector.tensor_scalar(own[:, :512],
                                                      psA[:, :512], 0.0, None,
                                                      op0=mybir.AluOpType.add)
                              nc.vector.tensor_scalar(
                                  own[:, F0:F0 + 2 * H0].bitcast(FP16),
                                  psB[:, :H0], 0.0, None,
                                  op0=mybir.AluOpType.add)
                      else:
                          if R0 > F0 + H0:
                              nc.vector.memset(own[:, F0 + H0:R0], 0.0)
                          if vb % 2 == 0:
                              nc.scalar.copy(own[:, :512], psA[:, :512])
                              nc.scalar.copy(own[:, 512:F0 + H0],
                                             psB[:, :F0 + H0 - 512])
                          else:
                              nc.vector.tensor_scalar(own[:, :512],
                                                      psA[:, :512], 0.0, None,
                                                      op0=mybir.AluOpType.add)
                              nc.vector.tensor_scalar(
                                  own[:, 512:F0 + H0],
                                  psB[:, :F0 + H0 - 512], 0.0,
                                  None, op0=mybir.AluOpType.add)
                  nc.sync.dma_start(
                      tables[0][vb0 * 128:(vb0 + XB) * 128, :]
                      .rearrange("(g p) r -> p g r", p=128),
                      ownb[:])
              # alpha_dst for OWN nodes (private xTo input): tiny projection of
              # just the 2H alpha columns per block.
              xto = cpool.tile([128, npad], FP16, tag="xto")
              nc.sync.dma_start(xto[:], xTo_in[:])
              for b in range(nblk):
                  psB = ppB.tile([128, 128], FP32, tag="gB")
                  nc.tensor.matmul(psB[:, :H0],
                                   xto[:, b * 128:(b + 1) * 128],
                                   w_sb[0][0][:, F0 + H0:F0 + 2 * H0],
                                   start=True, stop=True)
                  nc.scalar.copy(ad_own[:, b * 8:b * 8 + H0], psB[:, :H0])

              # ---------- layers
              for li, L in enumerate(layers):
                  F, H, C, R, NW = L["F"], L["H"], L["C"], L["R"], L["NW"]
                  fts = _ftiles(F)
                  Ln = layers[li + 1] if li < 3 else None

                  table = tables[li]
                  # upconvert the fp8 AG output into the fp16 gather table
                  if AG8 and li >= 1:
                      R8 = L["R8"]
                      for vt in range(0, nvblk, CVT):
                          nt = min(CVT, nvblk - vt)
                          t8c = cvtp.tile([128, CVT, R8], FP8, tag="cvt8")
                          nc.sync.dma_start(
                              t8c[:, :nt, :],
                              table8s[li][vt * 128:(vt + nt) * 128, :]
                              .rearrange("(g p) r -> p g r", p=128))
                          thc = cvtp.tile([128, CVT, R], FP16, tag="cvth")
                          nc.vector.memset(thc[:, :nt, F + H:R], 0.0)
                          if (vt // CVT) % 2 == 0:
                              nc.scalar.copy(thc[:, :nt, :F], t8c[:, :nt, :F])
                              nc.scalar.copy(
                                  thc[:, :nt, F:F + H],
                                  t8c[:, :nt, F:F + 2 * H].bitcast(FP16))
                          else:
                              nc.vector.tensor_scalar(
                                  thc[:, :nt, :F], t8c[:, :nt, :F], 0.0, None,
                                  op0=mybir.AluOpType.add)
                              nc.vector.tensor_scalar(
                                  thc[:, :nt, F:F + H],
                                  t8c[:, :nt, F:F + 2 * H].bitcast(FP16),
                                  0.0, None, op0=mybir.AluOpType.add)
                          nc.sync.dma_start(
                              table[vt * 128:(vt + nt) * 128, :]
                              .rearrange("(g p) r -> p g r", p=128),
                              thc[:, :nt, :])
                  zT_next = None
                  if li < 3:
                      zT_next = [zpool.tile([128, npad], FP16,
                                            tag=f"zt{(li + 1) % 2}_{t}",
                                            name=f"zt{li + 1}_{t}")
                                 for t in range(len(fts))]

                  mcolb = 0
                  for b in range(nblk):
                      psagg = ppG.tile([128, 512], FP32, tag="agg")
                      p_all = pool.tile([128, int(D[b]), 8], FP16, tag="pall")
                      amx = pool.tile([128, int(D[b]), H], FP16, tag="amx")
                      nc.vector.tensor_tensor(
                          out=amx[:],
                          in0=mask_sb[:, mcolb:mcolb + int(D[b])][:, :, None]
                              .broadcast_to([128, int(D[b]), H]),
                          in1=ad_own[:, b * 8:b * 8 + H][:, None, :]
                              .broadcast_to([128, int(D[b]), H]),
                          op=mybir.AluOpType.add,
                      )
                      mcolb += int(D[b])
                      for (cb, j0, jc, ic, mc) in chunks:
                          if cb != b:
                              continue
                          ni = 128 * jc
                          RT = L["R8"] if TB8 else R
                          G = gpool.tile([128, jc, RT], FP8 if TB8 else FP16,
                                         tag="G")
                          nc.gpsimd.dma_gather(
                              out_ap=G[:],
                              in_ap=table[:],
                              idxs_ap=idx_sb[:, ic:ic + ni // 16],
                              num_idxs=ni,
                              num_idxs_reg=ni,
                              elem_size=RT,
                              single_packet=False,
                              queue_num=qctr[0] % NQ,
                          )
                          qctr[0] += 1
                          gs = pool.tile([128, jc, H], FP16, tag="gs")
                          nc.vector.tensor_tensor(
                              out=gs[:],
                              in0=(G[:, :, F:F + 2 * H].bitcast(FP16)
                                   if TB8 else G[:, :, F:F + H]),
                              in1=amx[:, j0:j0 + jc, :],
                              op=mybir.AluOpType.add,
                          )
                          nc.vector.scalar_tensor_tensor(
                              out=gs[:], in0=gs[:], scalar=NEG_SLOPE, in1=gs[:],
                              op0=mybir.AluOpType.mult, op1=mybir.AluOpType.max,
                          )
                          nc.scalar.activation(p_all[:, j0:j0 + jc, :H], gs[:],
                                               mybir.ActivationFunctionType.Exp)
                          if "nopmul" in abl:
                              M = G
                          else:
                              M = mpool.tile([128, jc, F], FP16, tag="M")
                              nc.vector.tensor_tensor(
                                  out=M[:].rearrange("p j (c h) -> p j c h", h=H),
                                  in0=G[:, :, :F].rearrange("p j (c h) -> p j c h", h=H),
                                  in1=p_all[:, j0:j0 + jc, None, :H]
                                      .broadcast_to([128, jc, C, H]),
                                  op=mybir.AluOpType.mult,
                              )
                          for j in range(jc):
                              if "nomm" in abl and j0 + j > 0:
                                  continue
                              nc.tensor.matmul(
                                  psagg[:, :F], ident[:], M[:, j, :F],
                                  start=(j0 + j == 0),
                                  stop=(j0 + j == int(D[b]) - 1
                                        or "nomm" in abl),
                              )
                      denom = pool.tile([128, H], FP32, tag="denom")
                      nc.vector.tensor_reduce(
                          denom[:], p_all[:, :, :H].rearrange("p j h -> p h j"),
                          axis=mybir.AxisListType.X, op=mybir.AluOpType.add)
                      nc.vector.tensor_scalar(denom[:], denom[:], 1e-16, None,
                                              op0=mybir.AluOpType.add)
                      rden = pool.tile([128, H], FP32, tag="rden")
                      nc.vector.reciprocal(rden[:], denom[:])
                      if li == 3:
                          nc.vector.tensor_scalar(rden[:], rden[:], 1.0 / H, None,
                                                  op0=mybir.AluOpType.mult)
                          out5 = pool.tile([128, C, H], FP32, tag="out5")
                          nc.vector.tensor_tensor(
                              out=out5[:],
                              in0=psagg[:, :F].rearrange("p (c h) -> p c h", h=H),
                              in1=rden[:, None, :].broadcast_to([128, C, H]),
                              op=mybir.AluOpType.mult,
                          )
                          mean = pool.tile([128, C], FP32, tag="mean")
                          nc.vector.tensor_reduce(
                              mean[:], out5[:],
                              axis=mybir.AxisListType.X, op=mybir.AluOpType.add)
                          mx = pool.tile([128, 1], FP32, tag="mx")
                          nc.vector.tensor_reduce(mx[:], mean[:],
                                                  axis=mybir.AxisListType.X,
                                                  op=mybir.AluOpType.max)
                          negm = pool.tile([128, 1], FP32, tag="negm")
                          nc.vector.tensor_scalar(negm[:], mx[:], -1.0, None,
                                                  op0=mybir.AluOpType.mult)
                          esc = pool.tile([128, C], FP32, tag="esc")
                          esum = pool.tile([128, 1], FP32, tag="esum")
                          nc.scalar.activation(esc[:], mean[:],
                                               mybir.ActivationFunctionType.Exp,
                                               bias=negm[:], accum_out=esum[:])
                          lns = pool.tile([128, 1], FP32, tag="lns")
                          nc.scalar.activation(lns[:], esum[:],
                                               mybir.ActivationFunctionType.Ln)
                          off = pool.tile([128, 1], FP32, tag="off")
                          nc.vector.tensor_tensor(out=off[:], in0=negm[:],
                                                  in1=lns[:],
                                                  op=mybir.AluOpType.subtract)
                          fin40 = pool.tile([128, C], FP32, tag="fin40")
                          nc.vector.tensor_scalar(fin40[:], mean[:], off[:], None,
                                                  op0=mybir.AluOpType.add)
                          nc.sync.dma_start(out_dram[b * 128:(b + 1) * 128, :],
                                            fin40[:])
                      else:
                          zb = pool.tile([128, F], FP16, tag="zb")
                          nc.vector.tensor_tensor(
                              out=zb[:].rearrange("p (c h) -> p c h", h=H),
                              in0=psagg[:, :F].rearrange("p (c h) -> p c h", h=H),
                              in1=rden[:, None, :].broadcast_to([128, C, H]),
                              op=mybir.AluOpType.mult,
                          )
                          for ft, (fo, fw) in enumerate(fts):
                              psT = ppB.tile([128, 128], FP16, tag="tr")
                              nc.tensor.transpose(psT[:fw, :], zb[:, fo:fo + fw],
                                                  ident[:])
                              t32 = pool.tile([128, 128], FP32, tag="t32")
                              nc.scalar.activation(
                                  t32[:fw, :], psT[:fw, :],
                                  mybir.ActivationFunctionType.Identity,
                                  bias=bn_sb[li][1][:fw, ft:ft + 1],
                                  scale=bn_sb[li][0][:fw, ft:ft + 1],
                              )
                              nc.vector.scalar_tensor_tensor(
                                  out=zT_next[ft][:fw, b * 128:(b + 1) * 128],
                                  in0=t32[:fw, :], scalar=float(pw),
                                  in1=t32[:fw, :],
                                  op0=mybir.AluOpType.mult,
                                  op1=mybir.AluOpType.max,
                              )
                          # ---- next-layer projection for this block + bounce
                          Fn, Hn, Rn, NWn = Ln["F"], Ln["H"], Ln["R"], Ln["NW"]
                          ktsn = _ktiles(Ln["fin"])
                          psP = ppA.tile([128, 512], FP32, tag="gA")
                          last3 = (li + 1 == 3)
                          for kt, (o, k) in enumerate(ktsn):
                              nc.tensor.matmul(
                                  psP[:, :NWn],
                                  zT_next[kt][:k, b * 128:(b + 1) * 128],
                                  w_sb[li + 1][kt][:, :NWn],
                                  start=(kt == 0),
                                  stop=(kt == len(ktsn) - 1 and not last3),
                              )
                          if last3:
                              nc.tensor.matmul(psP[:, :NWn], ones1[:], b4row[:],
                                               start=False, stop=True)
                          if AG8 or TB8:
                              Rn8 = Ln["R8"]
                              ownn = pool.tile([128, Rn8], FP8, tag="ownn")
                              if Rn8 > Fn + 2 * Hn:
                                  nc.vector.memset(ownn[:, Fn + 2 * Hn:Rn8], 0.0)
                              nc.scalar.copy(ownn[:, :Fn], psP[:, :Fn])
                              nc.scalar.copy(
                                  ownn[:, Fn:Fn + 2 * Hn].bitcast(FP16),
                                  psP[:, Fn:Fn + Hn])
                          else:
                              ownn = pool.tile([128, Rn], FP16, tag="ownn")
                              if Rn > Fn + Hn:
                                  nc.vector.memset(ownn[:, Fn + Hn:Rn], 0.0)
                              nc.scalar.copy(ownn[:, :Fn + Hn], psP[:, :Fn + Hn])
                          nc.scalar.copy(ad_own[:, b * 8:b * 8 + Hn],
                                         psP[:, Fn + Hn:Fn + 2 * Hn])
                          nc.sync.dma_start(
                              bounces[li + 1][b * 128:(b + 1) * 128, :], ownn[:])
                          # fire the AllGather once all own blocks projected
                          if b == nblk - 1:
                              agdst = (table8s if AG8 else tables)[li + 1]
                              if cost_mode or "noag" in abl:
                                  nc.sync.dma_start(
                                      agdst[:npad, :],
                                      bounces[li + 1][:])
                              else:
                                  nc.gpsimd.collective_compute(
                                      "AllGather", mybir.AluOpType.bypass,
                                      replica_groups=[list(range(N_CORES))],
                                      ins=[bounces[li + 1][:].opt()],
                                      outs=[agdst[:].opt()],
                                  )
    nc.compile()
    return nc


# ------------------------------------------------------------------- kernel

def _np_reference(inputs):
    """Plain numpy port of the jax reference (for testing)."""
    x = inputs["x"].astype(np.float64)
    n = x.shape[0]
    e = inputs["edge_index"]
    src = np.concatenate([e[0], np.arange(n)]).astype(np.int64)
    dst = np.concatenate([e[1], np.arange(n)]).astype(np.int64)
    pw = float(np.asarray(inputs["pw"]).reshape(-1)[0])

    def gat(h_in, s, heads, out_c, concat):
        W = inputs["W" + s].astype(np.float64)
        a_s = inputs["as" + s].astype(np.float64)
        a_d = inputs["ad" + s].astype(np.float64)
        b = inputs["b" + s].astype(np.float64)
        h = (h_in @ W).reshape(n, heads, out_c)
        als = np.einsum("nhc,hc->nh", h, a_s)
        ald = np.einsum("nhc,hc->nh", h, a_d)
        ee = als[src] + ald[dst]
        ee = np.where(ee > 0, ee, NEG_SLOPE * ee)
        m = np.full((n, heads), -np.inf)
        np.maximum.at(m, dst, ee)
        m = np.where(np.isfinite(m), m, 0.0)
        p = np.exp(ee - m[dst])
        den = np.zeros((n, heads))
        np.add.at(den, dst, p)
        alpha = p / (den[dst] + 1e-16)
        out = np.zeros((n, heads, out_c))
        np.add.at(out, dst, alpha[:, :, None] * h[src])
        out = out.reshape(n, heads * out_c) if concat else out.mean(1)
        return out + b

    def bn(h, s):
        g = inputs["g" + s].astype(np.float64)
        be = inputs["be" + s].astype(np.float64)
        m = inputs["m" + s].astype(np.float64)
        v = inputs["v" + s].astype(np.float64)
        return (h - m) * (g / np.sqrt(v + BN_EPS)) + be

    prelu = lambda h: np.where(h > 0, h, pw * h)
    C = inputs["as1"].shape[1]
    h = gat(x, "1", 8, C, True)
    h = prelu(bn(h, "1"))
    h = gat(h, "2", 7, C, True)
    h = prelu(bn(h, "2"))
    h = gat(h, "3", 7, C, True)
    h = prelu(bn(h, "3"))
    h = gat(h, "4", 5, inputs["as4"].shape[1], False)
    h = h - h.max(1, keepdims=True)
    lse = np.log(np.exp(h).sum(1, keepdims=True))
    return (h - lse).astype(np.float32)


def _make_in_maps(inputs, graph, layers, npc, npad):
    x = inputs["x"]
    n = x.shape[0]
    V = N_CORES * npad
    shared = {
        "ident": np.eye(128, dtype=np.float16),
        "ones1": np.ones((1, 128), np.float16),
    }
    b4r = np.zeros((1, layers[3]["NW"]), np.float16)
    H4, C4 = layers[3]["H"], layers[3]["C"]
    b4r[0, :H4 * C4] = np.repeat(layers[3]["b4"], H4).astype(np.float16)
    shared["b4row"] = b4r
    for li, L in enumerate(layers):
        for kt, (o, k) in enumerate(_ktiles(L["fin"])):
            shared[f"w{li}_{kt}"] = np.ascontiguousarray(
                L["Wext"][o:o + k, :]).astype(np.float16)
        if li < 3:
            nft = len(_ftiles(L["F"]))
            sc = np.zeros((128, nft), np.float32)
            bi = np.zeros((128, nft), np.float32)
            for ft, (fo, fw) in enumerate(_ftiles(L["F"])):
                sc[:fw, ft] = L["bnscale"][fo:fo + fw]
                bi[:fw, ft] = L["bnbias"][fo:fo + fw]
            shared[f"bnsc{li}"] = sc
            shared[f"bnbi{li}"] = bi

    # xT: ALL nodes' features, columns ordered by vid (same for every core)
    xT = np.zeros((128, V), np.float16)
    g2v = graph["g2v"]
    xT[:, g2v[np.arange(n)]] = x.T.astype(np.float16)
    shared["xT"] = xT

    in_maps = []
    for c in range(N_CORES):
        xTo = np.zeros((128, npad), np.float16)
        xTo[:, :npc] = x[graph["perms"][c]].T.astype(np.float16)
        m = dict(shared)
        m["xTo"] = xTo
        m["idx"] = np.ascontiguousarray(graph["idx_imgs"][c])
        m["mask"] = np.ascontiguousarray(graph["masks"][c])
        in_maps.append(m)
    return in_maps


def kernel(_sim=False, **inputs):
    inputs = {k: np.asarray(v) for k, v in inputs.items()}
    x = inputs["x"]
    edge_index = inputs["edge_index"]
    n = x.shape[0]
    npc = n // N_CORES
    npad = ((npc + 127) // 128) * 128

    graph = _prep_graph(n, edge_index, npc, npad)
    layers = _prep_layers(inputs)
    pw = float(np.asarray(inputs["pw"]).reshape(-1)[0])

    nc = _build(n, npc, npad, layers, graph, pw)
    in_maps = _make_in_maps(inputs, graph, layers, npc, npad)

    if _sim:
        from concourse.bass_interp import MultiCoreSim
        sim = MultiCoreSim(nc, num_cores=N_CORES, trace=False,
                           require_finite=False, require_nnan=False)
        cores = list(sim.cores.values())
        for c in range(N_CORES):
            for name, arr in in_maps[c].items():
                cores[c].tensor(name)[:] = arr
        sim.simulate(check_with_hw=False)
        results = [{"out": np.array(cores[c].tensor("out"))}
                   for c in range(N_CORES)]
    else:
        res = bass_utils.run_bass_kernel_spmd(
            nc, in_maps, core_ids=list(range(N_CORES)))
        results = res.results

    ncls = results[0]["out"].shape[1]
    out = np.empty((n, ncls), np.float32)
    for c in range(N_CORES):
        out[graph["perms"][c]] = results[c]["out"][:npc]
    return out
